# revision 23
# baseline (speedup 1.0000x reference)
"""Trainium2 Bass kernel for nn_CNN_pre_LSTM (dense_cnn).

Reference computation per sample (L=24):
    h = relu(conv1d(x, w11, b11))    # 1 -> 8 ch, k=3, same pad
    h = relu(conv1d(h, w12, b12))    # 8 -> 8
    h = maxpool2(h)                  # L 24 -> 12
    h = relu(conv1d(h, w21, b21))    # 8 -> 16
    h = relu(conv1d(h, w22, b22))    # 16 -> 16
    h = maxpool2(h)                  # L 12 -> 6
    y = h.reshape(96) @ Wl.T + bl    # 96 -> 24

Mapping: pure data parallel over the fused (S*B) batch across 8 cores;
16384 samples per core. On chip, activations live as [feature, batch_tile]
(features on SBUF partitions, batch on the free dim); each conv layer is
ONE dense banded matmul per 128-row output block (host-built matrices,
l-major/c-minor feature order, halo-overlapped l-halves so there is no
PSUM accumulation anywhere). 18 matmuls per 1024-sample tile is the PE
floor: each layer needs ceil(outs/128) PSUM writes per 512-col chunk
(PSUM bank = 512 fp32 cols), and blockdiag K-packing tricks all fail
because the halo duplication pushes every layer's outputs past 128.

Perf model (measured via ntff profile):
  - PE streams 512 cols/matmul at 2.4 GHz (216 ns start-to-start,
    LDWEIGHTS hidden) in HAM k=8/8 windows, 1.2 GHz in k=4/8 windows.
    The HAM power manager alternates these in ~3.4-10.2 us windows
    (~55% full speed); effective PE cadence ~6.4-8 us/tile. This is the
    kernel's wall; fp8 DoubleRow would halve PE work but e4m3 numerics
    fail the 2e-2 gate (any single fp8-quantized activation tensor
    alone gives 2.4-4.2% rel err; all-fp8 gives 9%).
  - Engine op cost is per FREE COLUMN, independent of partition count:
    ACT ~1.11us per [*,1024] psum evac; DVE ~1.28us (no 2x mode on fp32
    psum reads); DVE fp16 sbuf max ~0.68us (2x); GPSIMD ~1.4us (software
    Q7, no PSUM port - can only do the sbuf-side pool maxes).
  - DMA issue costs ~0.63us on the issuing queue (HWDGE) -> all DMA
    issue lives on the otherwise-idle Sync queue (6/tile = 3.8us).
  - 9 psum evacuations/tile must split across ACT+DVE only. Dead ends,
    verified on hw/walrus this session: GPSIMD has no PSUM port AND
    walrus rejects TENSOR_TENSOR on Pool (ISA check), so gpsimd can do
    neither evacs nor maxes; DMA cannot touch PSUM; DMA accum_op=max is
    rejected ("DMACopy does not support max with Copy mode"), only via
    gpsimd swdge anyway (~1us/issue); partition_all_reduce is a full
    all-reduce, not pair-pooling; psum is bank-granular, 8 banks, so
    pspool bufs=4 x [*,1024] fp32 is the hard max and [*,2048] merged
    evac tiles would halve pipeline depth for -15% evac time (net loss).
    The balanced optimum is ACT 6 ops (c11 a+b, c12 a+b, c22 a+b,
    Relu+bias ~6.7us/tile) / DVE 7 ops (c21 a+b tensor_scalar, 4 maxes,
    linear bias ~6.6us/tile), which matches the PE's HAM-blended cadence.
  - pooled layers (conv12, conv22) emit parity-grouped blocks (even l at
    rows 0:48, odd l at rows 64:112). A small SBUF->SBUF DMA aligns the
    odd block's partitions (compute engines cannot shift partitions;
    only DMA can move data across partition ranges).
  - the device shows a bimodal GLOBAL clock state across runs: identical
    NEFFs measure ~141-147us or ~165-175us (every engine's busy time
    scales ~1.2x together). Treat cross-run deltas under ~5% as noise.

The input is pre-transposed/chunked on the host to [n_tiles, 24, NT] per
core (DRAM partition strides must stay <= 32KB; 64KB strides crash the
device), and the output is produced as [n_tiles, 24, NT] fp32 and
reassembled on the host. Weights ship as three blobs: a small first-conv
blob (own tile+semaphore so the first LDWEIGHTS is not gated on the full
248KB transfer), the rest, and the biases.
"""

import numpy as np

import concourse.bass as bass
import concourse.tile as tile
import concourse.mybir as mybir
from concourse import bacc
from concourse.bass_utils import run_bass_kernel_spmd

# ---------------------------------------------------------------- config
N_CORES = 8
S, B, L = 512, 256, 24
SB = S * B
CORE_N = SB // N_CORES  # 16384

# compute dtype for matmul operands / intermediate activations:
#   "fp16"  : float16 operands, fp32 PSUM accumulate, NT=1024
#   "fp32r" : fp32 bits, PE in float32r mode, NT=512
#   "fp32"  : exact fp32 (PE 4x slower), NT=512
COMPUTE = "fp16"

# engine assignment for psum evacuations ("act"|"dve") and pool maxes
# ("dve" = DVE tensor_max after an align DMA; "dma" = plain-copy DMA of the
# even block + gpsimd-issued accum_op=max DMA folding in the odd block, no
# compute-engine time at all)
EVAC = {"c11": "act", "c12": "act", "c21": "dve", "c22": "act", "lin": "dve"}
MAXES = {"c12": "dve", "c22": "dve"}
SKEWS = (0, 1, 3, 4, 5)
# emission order of stages within a step (indices into the stage list)
ORDER = (0, 1, 2, 3, 4)


def _cfg(compute):
    if compute == "fp16":
        return dict(dt=mybir.dt.float16, np_dt=np.float16, nt=1024, mm_cast=None)
    if compute == "fp32r":
        return dict(
            dt=mybir.dt.float32, np_dt=np.float32, nt=512, mm_cast=mybir.dt.float32r
        )
    if compute == "fp32":
        return dict(dt=mybir.dt.float32, np_dt=np.float32, nt=512, mm_cast=None)
    raise ValueError(compute)


# ------------------------------------------------- host weight transforms
#
# Feature row orderings (all l-major, c-minor):
#   h1 block A: rows (l, c)  l in [0,13), c in [0,8)   -> 104 rows
#   h1 block B: rows (l, c)  l in [11,24)              -> 104 rows
#   conv12 out (parity): rows par*64 + lp*8 + c        -> 112 rows used
#   pooled h2:  rows [lp 0..5 x8ch | 16 pad | lp 6..11 x8ch] = 112
#   h3 block A: rows (l, c16) l in [0,7)               -> 112 rows
#   h3 block B: rows (l-5, c16) l in [5,12)            -> 112 rows
#   conv22 out (parity): rows par*64 + lp*16 + c       -> 112 rows used
#   pooled h4:  rows [lp 0..2 x16ch | 16 pad | lp 3..5 x16ch] = 112
#   out: rows j in [0,24)

def _band_first(w, l_ins, l_outs, cin, cout):
    """Dense banded matrix [len(l_ins)*cin, len(l_outs)*cout] for a k=3
    'same' conv, rows (l_in, ci) l-major, cols (l_out, co) l-major."""
    K = len(l_ins) * cin
    M = len(l_outs) * cout
    W = np.zeros((K, M), np.float32)
    for ki, li in enumerate(l_ins):
        for ci in range(cin):
            for mo, lo in enumerate(l_outs):
                d = li - lo + 1
                if 0 <= d < 3:
                    for co in range(cout):
                        W[ki * cin + ci, mo * cout + co] = w[co, ci, d]
    return W


def _band_parity(w, l_ins, l_out_base, half_l, cin, cout):
    """Banded matrix with parity-grouped output: cols = par*64 + lp*cout +
    co, l_out = l_out_base + 2*lp + par (even block cols 0:48, odd block
    cols 64:112; pads 48:64 and 112:128 are zeroed by the matmul so the
    full [128] tensor is initialized)."""
    K = len(l_ins) * cin
    W = np.zeros((K, 128), np.float32)
    for ki, li in enumerate(l_ins):
        for ci in range(cin):
            for par in range(2):
                for lp in range(half_l):
                    lo = l_out_base + 2 * lp + par
                    d = li - lo + 1
                    if 0 <= d < 3:
                        for co in range(cout):
                            W[ki * cin + ci, par * 64 + lp * cout + co] = w[co, ci, d]
    return W


def _pad48(W):
    """Insert 16 zero rows at row 48 (pooled tensors carry a pad block)."""
    return np.concatenate([W[:48], np.zeros((16,) + W.shape[1:], W.dtype), W[48:]], 0)


def _host_weights(w11, b11, w12, b12, w21, b21, w22, b22, Wl, bl):
    f32 = np.float32
    w11, w12, w21, w22, Wl = (np.asarray(a, f32) for a in (w11, w12, w21, w22, Wl))

    W11A = _band_first(w11, range(0, 24), range(0, 13), 1, 8)      # [24, 104]
    W11B = _band_first(w11, range(0, 24), range(11, 24), 1, 8)     # [24, 104]
    W12A = _band_parity(w12, range(0, 13), 0, 6, 8, 8)             # [104, 128]
    W12B = _band_parity(w12, range(11, 24), 12, 6, 8, 8)           # [104, 128]
    W21A = _pad48(_band_first(w21, range(0, 12), range(0, 7), 8, 16))   # [112, 112]
    W21B = _pad48(_band_first(w21, range(0, 12), range(5, 12), 8, 16))  # [112, 112]
    W22A = _band_parity(w22, range(0, 7), 0, 3, 16, 16)            # [112, 128]
    W22B = _band_parity(w22, range(5, 12), 6, 3, 16, 16)           # [112, 128]
    # torch flatten feature = c*6 + lp ; h4 row = lp*16 + c (plus pad48)
    WLIN = np.zeros((96, 24), f32)
    for lp in range(6):
        for c in range(16):
            WLIN[lp * 16 + c, :] = Wl[:, c * 6 + lp]
    WLIN = _pad48(WLIN)                                            # [112, 24]

    return {
        "w11a": W11A, "w11b": W11B, "w12a": W12A, "w12b": W12B,
        "w21a": W21A, "w21b": W21B, "w22a": W22A, "w22b": W22B,
        "wlin": WLIN,
        "b11v": np.tile(np.asarray(b11, f32), 13).reshape(104, 1),
        "b12v": np.tile(np.asarray(b12, f32), 16).reshape(128, 1),
        "b21v": np.tile(np.asarray(b21, f32), 7).reshape(112, 1),
        "b22v": np.tile(np.asarray(b22, f32), 8).reshape(128, 1),
        "blv": np.asarray(bl, f32).reshape(24, 1),
    }


# weight blob layout: blob 1 = first-conv weights (small, gates the first
# LDWEIGHTS), blob 2 = the rest. (name, K, M) in packing order.
_WSPEC1 = [("w11a", 24, 104), ("w11b", 24, 104)]
_WSPEC2 = [
    ("w12a", 104, 128), ("w12b", 104, 128),
    ("w21a", 112, 112), ("w21b", 112, 112),
    ("w22a", 112, 128), ("w22b", 112, 128),
    ("wlin", 112, 24),
]
_WOFF = {}
_WBLOB = {}
for _spec, _bi in ((_WSPEC1, 1), (_WSPEC2, 2)):
    _off = 0
    for _n, _k, _m in _spec:
        _WOFF[_n] = _off
        _WBLOB[_n] = _bi
        _off += _m
W1_COLS = sum(m for _, _, m in _WSPEC1)
W2_COLS = sum(m for _, _, m in _WSPEC2)

_BSPEC = [("b11v", 104), ("b12v", 128), ("b21v", 112), ("b22v", 128), ("blv", 24)]
_BOFF = {n: i for i, (n, _) in enumerate(_BSPEC)}


def _pack_blobs(W, np_dt):
    wb1 = np.zeros((128, W1_COLS), np_dt)
    wb2 = np.zeros((128, W2_COLS), np_dt)
    for spec, wb in ((_WSPEC1, wb1), (_WSPEC2, wb2)):
        for n, k, m in spec:
            assert W[n].shape == (k, m), (n, W[n].shape)
            wb[:k, _WOFF[n]:_WOFF[n] + m] = W[n].astype(np_dt)
    bb = np.zeros((128, len(_BSPEC)), np.float32)
    for n, p in _BSPEC:
        bb[:p, _BOFF[n]] = W[n][:, 0]
    return wb1, wb2, bb


# ----------------------------------------------------- numpy device model
def emulate(x, np_dt=np.float16, **kw):
    """Pure-numpy emulation of the device dataflow (same banded matrices,
    same orderings, same cast points). Used to validate index math."""
    W = _host_weights(**kw)
    xt = np.ascontiguousarray(x.reshape(-1, L).T).astype(np_dt)  # [24, N]
    c = lambda a: a.astype(np_dt)

    def mm(wname, act):
        return c(W[wname]).astype(np.float32).T @ act.astype(np.float32)

    def relu_b(a, bias):
        return np.maximum(a + bias, 0.0)

    psA, psB = c(mm("w11a", xt)), c(mm("w11b", xt))
    h1a, h1b = c(relu_b(psA, W["b11v"])), c(relu_b(psB, W["b11v"]))
    psC, psD = c(mm("w12a", h1a)), c(mm("w12b", h1b))
    sA, sB = c(relu_b(psC, W["b12v"])), c(relu_b(psD, W["b12v"]))
    h2r = np.concatenate(
        [np.maximum(sA[0:64], sA[64:128]), np.maximum(sB[0:48], sB[64:112])], 0
    )
    psE, psF = c(mm("w21a", h2r)), c(mm("w21b", h2r))
    h3a, h3b = c(relu_b(psE, W["b21v"])), c(relu_b(psF, W["b21v"]))
    psG, psH = c(mm("w22a", h3a)), c(mm("w22b", h3b))
    sG, sH = c(relu_b(psG, W["b22v"])), c(relu_b(psH, W["b22v"]))
    h4r = np.concatenate(
        [np.maximum(sG[0:64], sG[64:128]), np.maximum(sH[0:48], sH[64:112])], 0
    )
    out = mm("wlin", h4r) + W["blv"]  # fp32
    return out.T.reshape(x.shape[0], x.shape[1], 24).astype(np.float32)


# --------------------------------------------------------- device builder
def build_kernel(n_samples, compute=COMPUTE, n_cores=N_CORES):
    cfg = _cfg(compute)
    DT, NT = cfg["dt"], cfg["nt"]
    MMC = cfg["mm_cast"]
    f32 = mybir.dt.float32
    n_tiles = n_samples // NT
    assert n_samples % NT == 0

    nc = bacc.Bacc(
        "TRN2",
        target_bir_lowering=False,
        debug=False,
        enable_asserts=False,
        num_devices=n_cores,
    )

    xt_d = nc.dram_tensor("xt", [n_tiles, 24, NT], DT, kind="ExternalInput").ap()
    w1_d = nc.dram_tensor("wblob1", [128, W1_COLS], DT, kind="ExternalInput").ap()
    w2_d = nc.dram_tensor("wblob2", [128, W2_COLS], DT, kind="ExternalInput").ap()
    bb_d = nc.dram_tensor("bblob", [128, len(_BSPEC)], f32,
                          kind="ExternalInput").ap()
    out_d = nc.dram_tensor("out", [n_tiles, 24, NT], f32, kind="ExternalOutput").ap()

    Relu = mybir.ActivationFunctionType.Relu
    Ident = mybir.ActivationFunctionType.Identity
    Add, Max = mybir.AluOpType.add, mybir.AluOpType.max

    ENG = {"act": nc.scalar, "dve": nc.vector, "gp": nc.gpsimd}

    def mmop(ap):
        return ap.bitcast(MMC) if MMC is not None else ap

    # matmul fp32 PSUM output must stay inside one 2KB bank -> <=512 cols
    MMN = min(NT, 512)

    with tile.TileContext(nc) as tc:
        with (
            tc.tile_pool(name="consts", bufs=1) as cpool,
            tc.tile_pool(name="xin", bufs=6) as xpool,
            tc.tile_pool(name="acts", bufs=5) as apool,
            tc.tile_pool(name="outs", bufs=3) as opool,
            tc.tile_pool(name="ps", bufs=4, space="PSUM") as pspool,
        ):
            warm = cpool.tile([1, 2], f32, tag="actwarm")
            nc.vector.memset(warm[:], 0.0)

            # prefetch order: the first matmul is gated only on xt(0) and
            # the small wblob1; bblob is needed by the first evacuation
            # ~1us later, wblob2 by the first conv12 matmul.
            xts = {}

            def prefetch(t):
                if t >= n_tiles:
                    return
                xt_t = xpool.tile([24, NT], DT, tag="xt")
                nc.sync.dma_start(xt_t[:], xt_d[t])
                xts[t] = xt_t

            w1sb = cpool.tile([128, W1_COLS], DT, tag="wblob1")
            w2sb = cpool.tile([128, W2_COLS], DT, tag="wblob2")
            bsb = cpool.tile([128, len(_BSPEC)], f32, tag="bblob")
            # parallelize head DMA issue across queues: xt0 on Sync, the
            # first-conv weights on ACT (issued before the warm ACTIVATE so
            # the issue precedes the ~1.3us table load on that queue),
            # biases on GpSimd (software DGE) -- serial issue on Sync alone
            # costs ~0.7us each and the first matmul is gated on xt0+wblob1
            nc.scalar.activation(warm[:], warm[:], Relu, bias=0.0)
            prefetch(0)
            nc.sync.dma_start(w1sb[:], w1_d)
            prefetch(1)
            nc.sync.dma_start(bsb[:], bb_d)
            nc.sync.dma_start(w2sb[:], w2_d)
            prefetch(2)

            def w(name):
                spec = _WSPEC1 if _WBLOB[name] == 1 else _WSPEC2
                k, m = next((kk, mm_) for nn, kk, mm_ in spec if nn == name)
                wsb = w1sb if _WBLOB[name] == 1 else w2sb
                return mmop(wsb[0:k, _WOFF[name]:_WOFF[name] + m])

            def bias(name):
                p = next(pp for nn, pp in _BSPEC if nn == name)
                return bsb[0:p, _BOFF[name]:_BOFF[name] + 1]

            def mm(out_ps, wname, rhs_sb):
                for j in range(0, NT, MMN):
                    nc.tensor.matmul(out_ps[:, j:j + MMN], w(wname),
                                     mmop(rhs_sb[:, j:j + MMN]),
                                     start=True, stop=True)

            def evac(key, dst, src, bname):
                e = EVAC[key]
                if e == "act":
                    nc.scalar.activation(dst, src, Relu, bias=bias(bname))
                else:
                    ENG[e].tensor_scalar(dst, src, bias(bname), 0.0, Add, Max)

            def pool(key, h2r, sa, sb):
                """h2r[0:64] = max(sa[0:64], sa[64:128]);
                h2r[64:112] = max(sb[64:112], sb[0:48])."""
                if MAXES[key] == "dma":
                    nc.sync.dma_start(h2r[0:64, :], sa[0:64, :])
                    nc.gpsimd.dma_start(h2r[0:64, :], sa[64:128, :],
                                        accum_op=Max)
                    nc.sync.dma_start(h2r[64:112, :], sb[64:112, :])
                    nc.gpsimd.dma_start(h2r[64:112, :], sb[0:48, :],
                                        accum_op=Max)
                else:
                    mv1 = apool.tile([64, NT], DT, tag="mv1")
                    mv2 = apool.tile([112, NT], DT, tag="mv2")
                    nc.sync.dma_start(mv1[0:64, :], sa[64:128, :])
                    nc.sync.dma_start(mv2[64:112, :], sb[0:48, :])
                    ENG[MAXES[key]].tensor_max(h2r[0:64, :], sa[0:64, :],
                                               mv1[0:64, :])
                    ENG[MAXES[key]].tensor_max(h2r[64:112, :], sb[64:112, :],
                                               mv2[64:112, :])

            # ---- software-pipelined emission -------------------------
            # Engines execute their instruction streams IN ORDER, so a
            # depth-first per-tile emission serializes tiles. Emitting the
            # five stages SKEWED across tiles interleaves independent work
            # in every engine's queue.
            h1 = {}
            h2 = {}
            h3 = {}
            h4 = {}

            def s1_conv11(t):
                if t in xts:
                    xt_t = xts.pop(t)
                else:
                    xt_t = xpool.tile([24, NT], DT, tag="xt")
                    nc.sync.dma_start(xt_t[:], xt_d[t])
                psA = pspool.tile([104, NT], f32, tag="ps")
                psB = pspool.tile([104, NT], f32, tag="ps")
                mm(psA, "w11a", xt_t)
                mm(psB, "w11b", xt_t)
                h1a = apool.tile([104, NT], DT, tag="h1a")
                h1b = apool.tile([104, NT], DT, tag="h1b")
                evac("c11", h1a[:], psA[:], "b11v")
                evac("c11", h1b[:], psB[:], "b11v")
                h1[t] = (h1a, h1b)

            def s2_conv12(t):
                h1a, h1b = h1.pop(t)
                psC = pspool.tile([128, NT], f32, tag="ps")
                psD = pspool.tile([128, NT], f32, tag="ps")
                mm(psC, "w12a", h1a)
                mm(psD, "w12b", h1b)
                s12a = apool.tile([128, NT], DT, tag="s12a")
                s12b = apool.tile([128, NT], DT, tag="s12b")
                evac("c12", s12a[:], psC[:], "b12v")
                evac("c12", s12b[:], psD[:], "b12v")
                h2r = apool.tile([112, NT], DT, tag="h2r")
                pool("c12", h2r, s12a, s12b)
                h2[t] = h2r

            def s3_conv21(t):
                h2r = h2.pop(t)
                psE = pspool.tile([112, NT], f32, tag="ps")
                psF = pspool.tile([112, NT], f32, tag="ps")
                mm(psE, "w21a", h2r)
                mm(psF, "w21b", h2r)
                h3a = apool.tile([112, NT], DT, tag="h3a")
                h3b = apool.tile([112, NT], DT, tag="h3b")
                evac("c21", h3a[:], psE[:], "b21v")
                evac("c21", h3b[:], psF[:], "b21v")
                h3[t] = (h3a, h3b)

            def s4_conv22(t):
                h3a, h3b = h3.pop(t)
                psG = pspool.tile([128, NT], f32, tag="ps")
                psH = pspool.tile([128, NT], f32, tag="ps")
                mm(psG, "w22a", h3a)
                mm(psH, "w22b", h3b)
                s22a = apool.tile([128, NT], DT, tag="s22a")
                s22b = apool.tile([128, NT], DT, tag="s22b")
                evac("c22", s22a[:], psG[:], "b22v")
                evac("c22", s22b[:], psH[:], "b22v")
                h4r = apool.tile([112, NT], DT, tag="h4r")
                pool("c22", h4r, s22a, s22b)
                h4[t] = h4r

            def s5_linear(t):
                h4r = h4.pop(t)
                psI = pspool.tile([24, NT], f32, tag="ps")
                mm(psI, "wlin", h4r)
                osb = opool.tile([24, NT], f32, tag="osb")
                e = EVAC["lin"]
                if e == "act":
                    nc.scalar.activation(osb[:], psI[:], Ident, bias=bias("blv"))
                else:
                    ENG[e].tensor_scalar_add(osb[:], psI[:], bias("blv"))
                nc.sync.dma_start(out_d[t], osb[:])

            # stage skews: each stage gets slack so the PE never idles
            # waiting on the pool chain (evac -> DMA align -> max).
            fns = [s1_conv11, s2_conv12, s3_conv21, s4_conv22, s5_linear]
            stages = [(SKEWS[i], fns[i]) for i in ORDER]
            for step in range(n_tiles + max(off for off, _ in stages)):
                for off, fn in stages:
                    t = step - off
                    if 0 <= t < n_tiles:
                        fn(t)

    nc.compile()
    return nc


# ------------------------------------------------------------- entry point
def _prep_in_maps(x, weights, compute=COMPUTE):
    cfg = _cfg(compute)
    np_dt = cfg["np_dt"]
    nt = cfg["nt"]
    W = _host_weights(**weights)
    wb1, wb2, bb = _pack_blobs(W, np_dt)
    xt = np.ascontiguousarray(x.reshape(SB, L).T).astype(np_dt)  # [24, SB]
    in_maps = []
    for c in range(N_CORES):
        xc = xt[:, c * CORE_N:(c + 1) * CORE_N]  # [24, CORE_N]
        in_maps.append({
            "xt": np.ascontiguousarray(
                xc.reshape(24, CORE_N // nt, nt).transpose(1, 0, 2)
            ),
            "wblob1": wb1,
            "wblob2": wb2,
            "bblob": bb,
        })
    return in_maps


def kernel(x, w11, b11, w12, b12, w21, b21, w22, b22, Wl, bl):
    weights = dict(w11=w11, b11=b11, w12=w12, b12=b12, w21=w21, b21=b21,
                   w22=w22, b22=b22, Wl=Wl, bl=bl)
    x = np.asarray(x, np.float32)
    nc = build_kernel(CORE_N, COMPUTE)
    in_maps = _prep_in_maps(x, weights, COMPUTE)
    res = run_bass_kernel_spmd(nc, in_maps, list(range(N_CORES)))
    outs = [
        res.results[c]["out"].transpose(1, 0, 2).reshape(24, CORE_N)
        for c in range(N_CORES)
    ]
    full = np.concatenate(outs, axis=1)  # [24, SB]
    return np.ascontiguousarray(full.T).reshape(S, B, 24).astype(np.float32)


# revision 24
# speedup vs baseline: 1.1716x; 1.1716x over previous
"""Trainium2 Bass kernel for nn_CNN_pre_LSTM (dense_cnn).

Reference computation per sample (L=24):
    h = relu(conv1d(x, w11, b11))    # 1 -> 8 ch, k=3, same pad
    h = relu(conv1d(h, w12, b12))    # 8 -> 8
    h = maxpool2(h)                  # L 24 -> 12
    h = relu(conv1d(h, w21, b21))    # 8 -> 16
    h = relu(conv1d(h, w22, b22))    # 16 -> 16
    h = maxpool2(h)                  # L 12 -> 6
    y = h.reshape(96) @ Wl.T + bl    # 96 -> 24

Mapping: pure data parallel over the fused (S*B) batch across 8 cores;
16384 samples per core. On chip, activations live as [feature, batch_tile]
(features on SBUF partitions, batch on the free dim); each conv layer is
ONE dense banded matmul per 128-row output block (host-built matrices,
l-major/c-minor feature order, halo-overlapped l-halves so there is no
PSUM accumulation anywhere). 18 matmuls per 1024-sample tile is the PE
floor: each layer needs ceil(outs/128) PSUM writes per 512-col chunk
(PSUM bank = 512 fp32 cols), and blockdiag K-packing tricks all fail
because the halo duplication pushes every layer's outputs past 128.

Perf model (measured via ntff profile):
  - PE streams 512 cols/matmul at 2.4 GHz (216 ns start-to-start,
    LDWEIGHTS hidden) in HAM k=8/8 windows, 1.2 GHz in k=4/8 windows.
    The HAM power manager alternates these in ~3.4-10.2 us windows
    (~55% full speed); effective PE cadence ~6.4-8 us/tile. This is the
    kernel's wall; fp8 DoubleRow would halve PE work but e4m3 numerics
    fail the 2e-2 gate (any single fp8-quantized activation tensor
    alone gives 2.4-4.2% rel err; all-fp8 gives 9%).
  - Engine op cost is per FREE COLUMN, independent of partition count:
    ACT ~1.11us per [*,1024] psum evac; DVE ~1.28us (no 2x mode on fp32
    psum reads); DVE fp16 sbuf max ~0.68us (2x); GPSIMD ~1.4us (software
    Q7, no PSUM port - can only do the sbuf-side pool maxes).
  - DMA issue costs ~0.63us on the issuing queue (hwdge = SP and ACT
    only; DVE cannot issue; gpsimd = software DGE ~1us) -> steady-state
    DMA issue lives on the otherwise-idle Sync queue (6/tile = 3.8us),
    and the head blob issues are spread across SP/ACT/GpSimd so the
    first matmul isn't gated on serial issue (paired A/B: ~1us faster).
  - 9 psum evacuations/tile must split across ACT+DVE only. Dead ends,
    verified on hw/walrus this session: GPSIMD has no PSUM port AND
    walrus rejects TENSOR_TENSOR on Pool (ISA check), so gpsimd can do
    neither evacs nor maxes; DMA cannot touch PSUM; DMA accum_op=max is
    rejected ("DMACopy does not support max with Copy mode"), only via
    gpsimd swdge anyway (~1us/issue); partition_all_reduce is a full
    all-reduce, not pair-pooling; psum is bank-granular, 8 banks, so
    pspool bufs=4 x [*,1024] fp32 is the hard max and [*,2048] merged
    evac tiles would halve pipeline depth for -15% evac time (net loss).
    The balanced optimum is ACT 6 ops (c11 a+b, c12 a+b, c22 a+b,
    Relu+bias ~6.7us/tile) / DVE 7 ops (c21 a+b tensor_scalar, 4 maxes,
    linear bias ~6.6us/tile), which matches the PE's HAM-blended cadence.
  - pooled layers (conv12, conv22) emit parity-grouped blocks (even l at
    rows 0:48, odd l at rows 64:112). A small SBUF->SBUF DMA aligns the
    odd block's partitions (compute engines cannot shift partitions;
    only DMA can move data across partition ranges).
  - the device shows a bimodal GLOBAL clock state across runs: identical
    NEFFs measure ~141-147us or ~165-175us (every engine's busy time
    scales ~1.2x together). Treat cross-run deltas under ~5% as noise.

The input is pre-transposed/chunked on the host to [n_tiles, 24, NT] per
core (DRAM partition strides must stay <= 32KB; 64KB strides crash the
device), and the output is produced as [n_tiles, 24, NT] fp32 and
reassembled on the host. Weights ship as three blobs: a small first-conv
blob (own tile+semaphore so the first LDWEIGHTS is not gated on the full
248KB transfer), the rest, and the biases.
"""

import numpy as np

import concourse.bass as bass
import concourse.tile as tile
import concourse.mybir as mybir
from concourse import bacc
from concourse.bass_utils import run_bass_kernel_spmd

# ---------------------------------------------------------------- config
N_CORES = 8
S, B, L = 512, 256, 24
SB = S * B
CORE_N = SB // N_CORES  # 16384

# compute dtype for matmul operands / intermediate activations:
#   "fp16"  : float16 operands, fp32 PSUM accumulate, NT=1024
#   "fp32r" : fp32 bits, PE in float32r mode, NT=512
#   "fp32"  : exact fp32 (PE 4x slower), NT=512
COMPUTE = "fp16"

# engine assignment for psum evacuations ("act"|"dve") and pool maxes
# ("dve" = DVE tensor_max after an align DMA; "dma" = plain-copy DMA of the
# even block + gpsimd-issued accum_op=max DMA folding in the odd block, no
# compute-engine time at all)
EVAC = {"c11": "act", "c12": "act", "c21": "dve", "c22": "act", "lin": "dve"}
MAXES = {"c12": "dve", "c22": "dve"}
SKEWS = (0, 1, 3, 4, 5)
# emission order of stages within a step (indices into the stage list)
ORDER = (0, 1, 2, 3, 4)


def _cfg(compute):
    if compute == "fp16":
        return dict(dt=mybir.dt.float16, np_dt=np.float16, nt=1024, mm_cast=None)
    if compute == "fp32r":
        return dict(
            dt=mybir.dt.float32, np_dt=np.float32, nt=512, mm_cast=mybir.dt.float32r
        )
    if compute == "fp32":
        return dict(dt=mybir.dt.float32, np_dt=np.float32, nt=512, mm_cast=None)
    raise ValueError(compute)


# ------------------------------------------------- host weight transforms
#
# Feature row orderings (all l-major, c-minor):
#   h1 block A: rows (l, c)  l in [0,13), c in [0,8)   -> 104 rows
#   h1 block B: rows (l, c)  l in [11,24)              -> 104 rows
#   conv12 out (parity): rows par*64 + lp*8 + c        -> 112 rows used
#   pooled h2:  rows [lp 0..5 x8ch | 16 pad | lp 6..11 x8ch] = 112
#   h3 block A: rows (l, c16) l in [0,7)               -> 112 rows
#   h3 block B: rows (l-5, c16) l in [5,12)            -> 112 rows
#   conv22 out (parity): rows par*64 + lp*16 + c       -> 112 rows used
#   pooled h4:  rows [lp 0..2 x16ch | 16 pad | lp 3..5 x16ch] = 112
#   out: rows j in [0,24)

def _band_first(w, l_ins, l_outs, cin, cout):
    """Dense banded matrix [len(l_ins)*cin, len(l_outs)*cout] for a k=3
    'same' conv, rows (l_in, ci) l-major, cols (l_out, co) l-major."""
    K = len(l_ins) * cin
    M = len(l_outs) * cout
    W = np.zeros((K, M), np.float32)
    for ki, li in enumerate(l_ins):
        for ci in range(cin):
            for mo, lo in enumerate(l_outs):
                d = li - lo + 1
                if 0 <= d < 3:
                    for co in range(cout):
                        W[ki * cin + ci, mo * cout + co] = w[co, ci, d]
    return W


def _band_parity(w, l_ins, l_out_base, half_l, cin, cout):
    """Banded matrix with parity-grouped output: cols = par*64 + lp*cout +
    co, l_out = l_out_base + 2*lp + par (even block cols 0:48, odd block
    cols 64:112; pads 48:64 and 112:128 are zeroed by the matmul so the
    full [128] tensor is initialized)."""
    K = len(l_ins) * cin
    W = np.zeros((K, 128), np.float32)
    for ki, li in enumerate(l_ins):
        for ci in range(cin):
            for par in range(2):
                for lp in range(half_l):
                    lo = l_out_base + 2 * lp + par
                    d = li - lo + 1
                    if 0 <= d < 3:
                        for co in range(cout):
                            W[ki * cin + ci, par * 64 + lp * cout + co] = w[co, ci, d]
    return W


def _pad48(W):
    """Insert 16 zero rows at row 48 (pooled tensors carry a pad block)."""
    return np.concatenate([W[:48], np.zeros((16,) + W.shape[1:], W.dtype), W[48:]], 0)


def _host_weights(w11, b11, w12, b12, w21, b21, w22, b22, Wl, bl):
    f32 = np.float32
    w11, w12, w21, w22, Wl = (np.asarray(a, f32) for a in (w11, w12, w21, w22, Wl))

    W11A = _band_first(w11, range(0, 24), range(0, 13), 1, 8)      # [24, 104]
    W11B = _band_first(w11, range(0, 24), range(11, 24), 1, 8)     # [24, 104]
    W12A = _band_parity(w12, range(0, 13), 0, 6, 8, 8)             # [104, 128]
    W12B = _band_parity(w12, range(11, 24), 12, 6, 8, 8)           # [104, 128]
    W21A = _pad48(_band_first(w21, range(0, 12), range(0, 7), 8, 16))   # [112, 112]
    W21B = _pad48(_band_first(w21, range(0, 12), range(5, 12), 8, 16))  # [112, 112]
    W22A = _band_parity(w22, range(0, 7), 0, 3, 16, 16)            # [112, 128]
    W22B = _band_parity(w22, range(5, 12), 6, 3, 16, 16)           # [112, 128]
    # torch flatten feature = c*6 + lp ; h4 row = lp*16 + c (plus pad48)
    WLIN = np.zeros((96, 24), f32)
    for lp in range(6):
        for c in range(16):
            WLIN[lp * 16 + c, :] = Wl[:, c * 6 + lp]
    WLIN = _pad48(WLIN)                                            # [112, 24]

    return {
        "w11a": W11A, "w11b": W11B, "w12a": W12A, "w12b": W12B,
        "w21a": W21A, "w21b": W21B, "w22a": W22A, "w22b": W22B,
        "wlin": WLIN,
        "b11v": np.tile(np.asarray(b11, f32), 13).reshape(104, 1),
        "b12v": np.tile(np.asarray(b12, f32), 16).reshape(128, 1),
        "b21v": np.tile(np.asarray(b21, f32), 7).reshape(112, 1),
        "b22v": np.tile(np.asarray(b22, f32), 8).reshape(128, 1),
        "blv": np.asarray(bl, f32).reshape(24, 1),
    }


# weight blob layout: blob 1 = first-conv weights (small, gates the first
# LDWEIGHTS), blob 2 = the rest. (name, K, M) in packing order.
_WSPEC1 = [("w11a", 24, 104), ("w11b", 24, 104)]
_WSPEC2 = [
    ("w12a", 104, 128), ("w12b", 104, 128),
    ("w21a", 112, 112), ("w21b", 112, 112),
    ("w22a", 112, 128), ("w22b", 112, 128),
    ("wlin", 112, 24),
]
_WOFF = {}
_WBLOB = {}
for _spec, _bi in ((_WSPEC1, 1), (_WSPEC2, 2)):
    _off = 0
    for _n, _k, _m in _spec:
        _WOFF[_n] = _off
        _WBLOB[_n] = _bi
        _off += _m
W1_COLS = sum(m for _, _, m in _WSPEC1)
W2_COLS = sum(m for _, _, m in _WSPEC2)

_BSPEC = [("b11v", 104), ("b12v", 128), ("b21v", 112), ("b22v", 128), ("blv", 24)]
_BOFF = {n: i for i, (n, _) in enumerate(_BSPEC)}


def _pack_blobs(W, np_dt):
    wb1 = np.zeros((128, W1_COLS), np_dt)
    wb2 = np.zeros((128, W2_COLS), np_dt)
    for spec, wb in ((_WSPEC1, wb1), (_WSPEC2, wb2)):
        for n, k, m in spec:
            assert W[n].shape == (k, m), (n, W[n].shape)
            wb[:k, _WOFF[n]:_WOFF[n] + m] = W[n].astype(np_dt)
    bb = np.zeros((128, len(_BSPEC)), np.float32)
    for n, p in _BSPEC:
        bb[:p, _BOFF[n]] = W[n][:, 0]
    return wb1, wb2, bb


# ----------------------------------------------------- numpy device model
def emulate(x, np_dt=np.float16, **kw):
    """Pure-numpy emulation of the device dataflow (same banded matrices,
    same orderings, same cast points). Used to validate index math."""
    W = _host_weights(**kw)
    xt = np.ascontiguousarray(x.reshape(-1, L).T).astype(np_dt)  # [24, N]
    c = lambda a: a.astype(np_dt)

    def mm(wname, act):
        return c(W[wname]).astype(np.float32).T @ act.astype(np.float32)

    def relu_b(a, bias):
        return np.maximum(a + bias, 0.0)

    psA, psB = c(mm("w11a", xt)), c(mm("w11b", xt))
    h1a, h1b = c(relu_b(psA, W["b11v"])), c(relu_b(psB, W["b11v"]))
    psC, psD = c(mm("w12a", h1a)), c(mm("w12b", h1b))
    sA, sB = c(relu_b(psC, W["b12v"])), c(relu_b(psD, W["b12v"]))
    h2r = np.concatenate(
        [np.maximum(sA[0:64], sA[64:128]), np.maximum(sB[0:48], sB[64:112])], 0
    )
    psE, psF = c(mm("w21a", h2r)), c(mm("w21b", h2r))
    h3a, h3b = c(relu_b(psE, W["b21v"])), c(relu_b(psF, W["b21v"]))
    psG, psH = c(mm("w22a", h3a)), c(mm("w22b", h3b))
    sG, sH = c(relu_b(psG, W["b22v"])), c(relu_b(psH, W["b22v"]))
    h4r = np.concatenate(
        [np.maximum(sG[0:64], sG[64:128]), np.maximum(sH[0:48], sH[64:112])], 0
    )
    out = mm("wlin", h4r) + W["blv"]  # fp32
    return out.T.reshape(x.shape[0], x.shape[1], 24).astype(np.float32)


# --------------------------------------------------------- device builder
def build_kernel(n_samples, compute=COMPUTE, n_cores=N_CORES):
    cfg = _cfg(compute)
    DT, NT = cfg["dt"], cfg["nt"]
    MMC = cfg["mm_cast"]
    f32 = mybir.dt.float32
    n_tiles = n_samples // NT
    assert n_samples % NT == 0

    nc = bacc.Bacc(
        "TRN2",
        target_bir_lowering=False,
        debug=False,
        enable_asserts=False,
        num_devices=n_cores,
    )

    xt_d = nc.dram_tensor("xt", [n_tiles, 24, NT], DT, kind="ExternalInput").ap()
    w1_d = nc.dram_tensor("wblob1", [128, W1_COLS], DT, kind="ExternalInput").ap()
    w2_d = nc.dram_tensor("wblob2", [128, W2_COLS], DT, kind="ExternalInput").ap()
    bb_d = nc.dram_tensor("bblob", [128, len(_BSPEC)], f32,
                          kind="ExternalInput").ap()
    out_d = nc.dram_tensor("out", [n_tiles, 24, NT], f32, kind="ExternalOutput").ap()

    Relu = mybir.ActivationFunctionType.Relu
    Ident = mybir.ActivationFunctionType.Identity
    Add, Max = mybir.AluOpType.add, mybir.AluOpType.max

    ENG = {"act": nc.scalar, "dve": nc.vector, "gp": nc.gpsimd}

    def mmop(ap):
        return ap.bitcast(MMC) if MMC is not None else ap

    # matmul fp32 PSUM output must stay inside one 2KB bank -> <=512 cols
    MMN = min(NT, 512)

    with tile.TileContext(nc) as tc:
        with (
            tc.tile_pool(name="consts", bufs=1) as cpool,
            tc.tile_pool(name="xin", bufs=6) as xpool,
            tc.tile_pool(name="acts", bufs=5) as apool,
            tc.tile_pool(name="outs", bufs=3) as opool,
            tc.tile_pool(name="ps", bufs=4, space="PSUM") as pspool,
        ):
            warm = cpool.tile([1, 2], f32, tag="actwarm")
            nc.vector.memset(warm[:], 0.0)

            # prefetch order: the first matmul is gated only on xt(0) and
            # the small wblob1; bblob is needed by the first evacuation
            # ~1us later, wblob2 by the first conv12 matmul.
            xts = {}

            def prefetch(t):
                if t >= n_tiles:
                    return
                xt_t = xpool.tile([24, NT], DT, tag="xt")
                nc.sync.dma_start(xt_t[:], xt_d[t])
                xts[t] = xt_t

            w1sb = cpool.tile([128, W1_COLS], DT, tag="wblob1")
            w2sb = cpool.tile([128, W2_COLS], DT, tag="wblob2")
            bsb = cpool.tile([128, len(_BSPEC)], f32, tag="bblob")
            # parallelize head DMA issue across queues: xt0 on Sync, the
            # first-conv weights on ACT (issued before the warm ACTIVATE so
            # the issue precedes the ~1.3us table load on that queue),
            # biases on GpSimd (software DGE) -- serial issue on Sync alone
            # costs ~0.7us each and the first matmul is gated on xt0+wblob1
            prefetch(0)
            nc.scalar.dma_start(w1sb[:], w1_d)
            nc.gpsimd.dma_start(bsb[:], bb_d)
            # prefetch the ACT spline-table set (~2.7us) during the blob
            # DMAs: a dummy ACTIVATE forces walrus to place the table load
            # at the head of ACT's stream instead of before tile 0's evac
            nc.scalar.activation(warm[:], warm[:], Relu, bias=0.0)
            prefetch(1)
            nc.sync.dma_start(w2sb[:], w2_d)
            prefetch(2)

            def w(name):
                spec = _WSPEC1 if _WBLOB[name] == 1 else _WSPEC2
                k, m = next((kk, mm_) for nn, kk, mm_ in spec if nn == name)
                wsb = w1sb if _WBLOB[name] == 1 else w2sb
                return mmop(wsb[0:k, _WOFF[name]:_WOFF[name] + m])

            def bias(name):
                p = next(pp for nn, pp in _BSPEC if nn == name)
                return bsb[0:p, _BOFF[name]:_BOFF[name] + 1]

            def mm(out_ps, wname, rhs_sb):
                for j in range(0, NT, MMN):
                    nc.tensor.matmul(out_ps[:, j:j + MMN], w(wname),
                                     mmop(rhs_sb[:, j:j + MMN]),
                                     start=True, stop=True)

            def evac(key, dst, src, bname):
                e = EVAC[key]
                if e == "act":
                    nc.scalar.activation(dst, src, Relu, bias=bias(bname))
                else:
                    ENG[e].tensor_scalar(dst, src, bias(bname), 0.0, Add, Max)

            def pool(key, h2r, sa, sb):
                """h2r[0:64] = max(sa[0:64], sa[64:128]);
                h2r[64:112] = max(sb[64:112], sb[0:48])."""
                if MAXES[key] == "dma":
                    nc.sync.dma_start(h2r[0:64, :], sa[0:64, :])
                    nc.gpsimd.dma_start(h2r[0:64, :], sa[64:128, :],
                                        accum_op=Max)
                    nc.sync.dma_start(h2r[64:112, :], sb[64:112, :])
                    nc.gpsimd.dma_start(h2r[64:112, :], sb[0:48, :],
                                        accum_op=Max)
                else:
                    mv1 = apool.tile([64, NT], DT, tag="mv1")
                    mv2 = apool.tile([112, NT], DT, tag="mv2")
                    nc.sync.dma_start(mv1[0:64, :], sa[64:128, :])
                    nc.sync.dma_start(mv2[64:112, :], sb[0:48, :])
                    ENG[MAXES[key]].tensor_max(h2r[0:64, :], sa[0:64, :],
                                               mv1[0:64, :])
                    ENG[MAXES[key]].tensor_max(h2r[64:112, :], sb[64:112, :],
                                               mv2[64:112, :])

            # ---- software-pipelined emission -------------------------
            # Engines execute their instruction streams IN ORDER, so a
            # depth-first per-tile emission serializes tiles. Emitting the
            # five stages SKEWED across tiles interleaves independent work
            # in every engine's queue.
            h1 = {}
            h2 = {}
            h3 = {}
            h4 = {}

            def s1_conv11(t):
                if t in xts:
                    xt_t = xts.pop(t)
                else:
                    xt_t = xpool.tile([24, NT], DT, tag="xt")
                    nc.sync.dma_start(xt_t[:], xt_d[t])
                psA = pspool.tile([104, NT], f32, tag="ps")
                psB = pspool.tile([104, NT], f32, tag="ps")
                mm(psA, "w11a", xt_t)
                mm(psB, "w11b", xt_t)
                h1a = apool.tile([104, NT], DT, tag="h1a")
                h1b = apool.tile([104, NT], DT, tag="h1b")
                evac("c11", h1a[:], psA[:], "b11v")
                evac("c11", h1b[:], psB[:], "b11v")
                h1[t] = (h1a, h1b)

            def s2_conv12(t):
                h1a, h1b = h1.pop(t)
                psC = pspool.tile([128, NT], f32, tag="ps")
                psD = pspool.tile([128, NT], f32, tag="ps")
                mm(psC, "w12a", h1a)
                mm(psD, "w12b", h1b)
                s12a = apool.tile([128, NT], DT, tag="s12a")
                s12b = apool.tile([128, NT], DT, tag="s12b")
                evac("c12", s12a[:], psC[:], "b12v")
                evac("c12", s12b[:], psD[:], "b12v")
                h2r = apool.tile([112, NT], DT, tag="h2r")
                pool("c12", h2r, s12a, s12b)
                h2[t] = h2r

            def s3_conv21(t):
                h2r = h2.pop(t)
                psE = pspool.tile([112, NT], f32, tag="ps")
                psF = pspool.tile([112, NT], f32, tag="ps")
                mm(psE, "w21a", h2r)
                mm(psF, "w21b", h2r)
                h3a = apool.tile([112, NT], DT, tag="h3a")
                h3b = apool.tile([112, NT], DT, tag="h3b")
                evac("c21", h3a[:], psE[:], "b21v")
                evac("c21", h3b[:], psF[:], "b21v")
                h3[t] = (h3a, h3b)

            def s4_conv22(t):
                h3a, h3b = h3.pop(t)
                psG = pspool.tile([128, NT], f32, tag="ps")
                psH = pspool.tile([128, NT], f32, tag="ps")
                mm(psG, "w22a", h3a)
                mm(psH, "w22b", h3b)
                s22a = apool.tile([128, NT], DT, tag="s22a")
                s22b = apool.tile([128, NT], DT, tag="s22b")
                evac("c22", s22a[:], psG[:], "b22v")
                evac("c22", s22b[:], psH[:], "b22v")
                h4r = apool.tile([112, NT], DT, tag="h4r")
                pool("c22", h4r, s22a, s22b)
                h4[t] = h4r

            def s5_linear(t):
                h4r = h4.pop(t)
                psI = pspool.tile([24, NT], f32, tag="ps")
                mm(psI, "wlin", h4r)
                osb = opool.tile([24, NT], f32, tag="osb")
                e = EVAC["lin"]
                if e == "act":
                    nc.scalar.activation(osb[:], psI[:], Ident, bias=bias("blv"))
                else:
                    ENG[e].tensor_scalar_add(osb[:], psI[:], bias("blv"))
                nc.sync.dma_start(out_d[t], osb[:])

            # stage skews: each stage gets slack so the PE never idles
            # waiting on the pool chain (evac -> DMA align -> max).
            fns = [s1_conv11, s2_conv12, s3_conv21, s4_conv22, s5_linear]
            stages = [(SKEWS[i], fns[i]) for i in ORDER]
            for step in range(n_tiles + max(off for off, _ in stages)):
                for off, fn in stages:
                    t = step - off
                    if 0 <= t < n_tiles:
                        fn(t)

    nc.compile()
    return nc


# ------------------------------------------------------------- entry point
def _prep_in_maps(x, weights, compute=COMPUTE):
    cfg = _cfg(compute)
    np_dt = cfg["np_dt"]
    nt = cfg["nt"]
    W = _host_weights(**weights)
    wb1, wb2, bb = _pack_blobs(W, np_dt)
    xt = np.ascontiguousarray(x.reshape(SB, L).T).astype(np_dt)  # [24, SB]
    in_maps = []
    for c in range(N_CORES):
        xc = xt[:, c * CORE_N:(c + 1) * CORE_N]  # [24, CORE_N]
        in_maps.append({
            "xt": np.ascontiguousarray(
                xc.reshape(24, CORE_N // nt, nt).transpose(1, 0, 2)
            ),
            "wblob1": wb1,
            "wblob2": wb2,
            "bblob": bb,
        })
    return in_maps


def kernel(x, w11, b11, w12, b12, w21, b21, w22, b22, Wl, bl):
    weights = dict(w11=w11, b11=b11, w12=w12, b12=b12, w21=w21, b21=b21,
                   w22=w22, b22=b22, Wl=Wl, bl=bl)
    x = np.asarray(x, np.float32)
    nc = build_kernel(CORE_N, COMPUTE)
    in_maps = _prep_in_maps(x, weights, COMPUTE)
    res = run_bass_kernel_spmd(nc, in_maps, list(range(N_CORES)))
    outs = [
        res.results[c]["out"].transpose(1, 0, 2).reshape(24, CORE_N)
        for c in range(N_CORES)
    ]
    full = np.concatenate(outs, axis=1)  # [24, SB]
    return np.ascontiguousarray(full.T).reshape(S, B, 24).astype(np.float32)
